# revision 19
# baseline (speedup 1.0000x reference)
"""Trainium2 Bass kernel for nn_BilateralFilter (exact Gaussian bilateral filter).

Math (per reference):
  feats f_i in R^6 (scaled spatial zyx + scaled rgb), N = 12*24*24 = 6912
  sq[i,j] = |f_i - f_j|^2 ;  K = exp(-0.5*sq)
  out[c,j] = (sum_i q[c,i] K[i,j]) / (sum_i K[i,j] + eps)

Device strategy (8 cores, row-sharded over the N x N kernel):
  Each core owns J = N/8 = 864 output columns j.  One PE matmul per 128-i
  tile computes arg = f_i.f_j - 0.5|f_i|^2 - 0.5|f_j|^2 = -0.5*sq directly
  in PSUM via an augmented-feature contraction, ScalarE exponentiates
  PSUM->SBUF, and a second PE matmul contracts K against stacked
  [q_hi, q_lo, ones] columns accumulating (filtered_hi, filtered_lo, norm)
  in PSUM.  The N x N kernel matrix never touches HBM.

  Default scheme "bf16" keeps fp32-class accuracy on the fast 1-cycle/row
  PE path by stacking hi/lo splits into the contraction dimension:
  matmul1 is bf16 with 3-level operand splits (6 block-pairs, exact
  cross-products, dropped terms ~2^-27), zero-padded to K=128 so the PE's
  fast-weight-load path stays enabled; matmul2 is fp16 with 2-level q
  splits, and the fp16 rounding of K largely cancels between the filtered
  numerator and the norm.  Max rel err vs the fp32 reference: ~2.4e-5
  (BILATERAL_MM_SCHEME=fp32 selects the exact-fp32 path, ~1e-6, ~3x slower;
  fp32r selects a float32r 5-pass split variant, ~1e-5).

Host does only O(N) layout/prep: the augmented/split feature matrices and
the final (2 x N) hi+lo add + normalization divide (reference eps semantics).
"""

import os
import numpy as np

try:
    import concourse.bass as bass
except ImportError:  # fresh grading dir: repo not on sys.path
    import sys

    sys.path.insert(0, "/opt/trn_rl_repo")
    import concourse.bass as bass

import concourse.mybir as mybir
import concourse.tile as tile
from concourse import bacc
from concourse.bass_utils import run_bass_kernel_spmd

SIGMA_ALPHA = (5.0, 5.0, 5.0)
SIGMA_BETA = 0.3
EPS = float(np.finfo("float").eps)

D, H, W = 12, 24, 24
N = D * H * W  # 6912
M_CORES = 8
J = N // M_CORES  # 864 output columns per core
NT = N // 128  # 54 i-tiles
F = 8  # augmented feature dim
J_CHUNKS = [(0, 512), (512, 864)]  # matmul free-dim chunks, PSUM-bank aligned

# Schemes:
#   fp32  — exact fp32 matmuls (4 cycles/row on PE, 2 HW passes each)
#   fp32r — TF32-style float32r matmuls (1 cycle/row) with hi/lo 10-bit
#           operand splits so the math stays fp32-accurate:
#             matmul1: Ah.Bh + Ah.Bl + Al.Bh   (drops Al.Bl ~ 5e-6 in arg)
#             matmul2: [q_hi,1] pass + [q_lo,0] pass; the HW's 10-bit
#             truncation of K cancels between numerator and norm.
_MM_SCHEME = os.environ.get("BILATERAL_MM_SCHEME", "bf16")

_BUILD_CACHE: dict[str, object] = {}


def _build_nc_fp32():
    nc = bacc.Bacc(None, target_bir_lowering=False)

    a_dram = nc.dram_tensor("a_all", [F, N], mybir.dt.float32, kind="ExternalInput")
    b_dram = nc.dram_tensor("b_slab", [F, J], mybir.dt.float32, kind="ExternalInput")
    qa_dram = nc.dram_tensor("qa", [N, 3], mybir.dt.float32, kind="ExternalInput")
    out_dram = nc.dram_tensor("acc_out", [3, J], mybir.dt.float32, kind="ExternalOutput")

    with tile.TileContext(nc) as tc:
        with (
            tc.tile_pool(name="const", bufs=1) as const_pool,
            tc.tile_pool(name="kpool", bufs=4) as kpool,
            tc.tile_pool(name="gpsum", bufs=2, space="PSUM") as gpool,
            tc.tile_pool(name="apsum", bufs=1, space="PSUM") as apool,
            tc.tile_pool(name="opool", bufs=1) as opool,
        ):
            A = const_pool.tile([F, N], mybir.dt.float32)
            B = const_pool.tile([F, J], mybir.dt.float32)
            QA = const_pool.tile([128, NT * 3], mybir.dt.float32)
            nc.sync.dma_start(A[:], a_dram[:])
            nc.sync.dma_start(B[:], b_dram[:])
            nc.sync.dma_start(
                QA[:].rearrange("p (t c) -> p t c", c=3),
                qa_dram[:].rearrange("(t p) c -> p t c", p=128),
            )

            acc = apool.tile([3, J], mybir.dt.float32)
            for t in range(NT):
                g = gpool.tile([128, J], mybir.dt.float32)
                for j0, j1 in J_CHUNKS:
                    nc.tensor.matmul(
                        g[:, j0:j1],
                        A[:, t * 128 : (t + 1) * 128],
                        B[:, j0:j1],
                        start=True,
                        stop=True,
                    )
                k = kpool.tile([128, J], mybir.dt.float32)
                nc.scalar.activation(k[:], g[:], mybir.ActivationFunctionType.Exp)
                for j0, j1 in J_CHUNKS:
                    nc.tensor.matmul(
                        acc[:, j0:j1],
                        QA[:, t * 3 : t * 3 + 3],
                        k[:, j0:j1],
                        start=(t == 0),
                        stop=(t == NT - 1),
                    )

            out_sb = opool.tile([3, J], mybir.dt.float32)
            nc.vector.tensor_copy(out_sb[:], acc[:])
            nc.sync.dma_start(out_dram[:], out_sb[:])

    nc.compile()
    return nc


def _build_nc_fp32r():
    f32r = mybir.dt.float32r
    nc = bacc.Bacc(None, target_bir_lowering=False)

    ins = {}
    for name, shape in [
        ("a_hi", [F, N]),
        ("a_lo", [F, N]),
        ("b_hi", [F, J]),
        ("b_lo", [F, J]),
        ("qa_hi", [N, 3]),
        ("qa_lo", [N, 3]),
    ]:
        ins[name] = nc.dram_tensor(name, shape, f32r, kind="ExternalInput")
    out_dram = nc.dram_tensor("acc_out", [3, J], mybir.dt.float32, kind="ExternalOutput")

    with tile.TileContext(nc) as tc:
        with (
            tc.tile_pool(name="const", bufs=1) as const_pool,
            tc.tile_pool(name="kpool", bufs=4) as kpool,
            tc.tile_pool(name="gpsum", bufs=2, space="PSUM") as gpool,
            tc.tile_pool(name="apsum", bufs=1, space="PSUM") as apool,
            tc.tile_pool(name="opool", bufs=1) as opool,
        ):
            Ah = const_pool.tile([F, N], f32r)
            Al = const_pool.tile([F, N], f32r)
            Bh = const_pool.tile([F, J], f32r)
            Bl = const_pool.tile([F, J], f32r)
            QAh = const_pool.tile([128, NT * 3], f32r)
            QAl = const_pool.tile([128, NT * 3], f32r)
            nc.sync.dma_start(Ah[:], ins["a_hi"][:])
            nc.sync.dma_start(Al[:], ins["a_lo"][:])
            nc.sync.dma_start(Bh[:], ins["b_hi"][:])
            nc.sync.dma_start(Bl[:], ins["b_lo"][:])
            for t_sb, t_dr in [(QAh, "qa_hi"), (QAl, "qa_lo")]:
                nc.sync.dma_start(
                    t_sb[:].rearrange("p (t c) -> p t c", c=3),
                    ins[t_dr][:].rearrange("(t p) c -> p t c", p=128),
                )

            acc = apool.tile([3, J], mybir.dt.float32)
            for t in range(NT):
                ts = slice(t * 128, (t + 1) * 128)
                g = gpool.tile([128, J], mybir.dt.float32)
                # Ah.Bh, Ah.Bl with the same stationary operand, then Al.Bh.
                for j0, j1 in J_CHUNKS:
                    nc.tensor.matmul(g[:, j0:j1], Ah[:, ts], Bh[:, j0:j1], start=True, stop=False)
                for j0, j1 in J_CHUNKS:
                    nc.tensor.matmul(g[:, j0:j1], Ah[:, ts], Bl[:, j0:j1], start=False, stop=False)
                for j0, j1 in J_CHUNKS:
                    nc.tensor.matmul(g[:, j0:j1], Al[:, ts], Bh[:, j0:j1], start=False, stop=True)
                k = kpool.tile([128, J], f32r)
                nc.scalar.activation(k[:], g[:], mybir.ActivationFunctionType.Exp)
                cs = slice(t * 3, t * 3 + 3)
                for j0, j1 in J_CHUNKS:
                    nc.tensor.matmul(
                        acc[:, j0:j1], QAh[:, cs], k[:, j0:j1],
                        start=(t == 0), stop=False,
                    )
                for j0, j1 in J_CHUNKS:
                    nc.tensor.matmul(
                        acc[:, j0:j1], QAl[:, cs], k[:, j0:j1],
                        start=False, stop=(t == NT - 1),
                    )

            out_sb = opool.tile([3, J], mybir.dt.float32)
            nc.vector.tensor_copy(out_sb[:], acc[:])
            nc.sync.dma_start(out_dram[:], out_sb[:])

    nc.compile()
    return nc


def _build_nc_bf16():
    """Stacked-split scheme: matmul1 bf16 K=48, matmul2 fp16 M=5 — one MM per
    PSUM chunk, standalone LDWEIGHTS + FWL + back-to-back PE streaming."""
    bf16 = mybir.dt.bfloat16
    f16 = mybir.dt.float16
    nc = bacc.Bacc(None, target_bir_lowering=False)

    a_dram = nc.dram_tensor("a48", [128, N], bf16, kind="ExternalInput")
    b_dram = nc.dram_tensor("b48", [128, J], bf16, kind="ExternalInput")
    qa_dram = nc.dram_tensor("qa5", [N, 5], f16, kind="ExternalInput")
    out_dram = nc.dram_tensor("acc_out", [5, J], mybir.dt.float32, kind="ExternalOutput")

    with tile.TileContext(nc) as tc:
        with (
            tc.tile_pool(name="const", bufs=1) as const_pool,
            tc.tile_pool(name="kpool", bufs=4) as kpool,
            tc.tile_pool(name="gpsum", bufs=3, space="PSUM") as gpool,
            tc.tile_pool(name="apsum", bufs=1, space="PSUM") as apool,
            tc.tile_pool(name="opool", bufs=1) as opool,
        ):
            A48 = const_pool.tile([128, N], bf16)
            B48 = const_pool.tile([128, J], bf16)
            QA5 = const_pool.tile([128, NT * 5], f16)
            # Column-split the loads so the first matmuls only wait for the
            # slices they need (tile 0 needs A cols 0:128 + B chunk 0), not
            # the full 1.8 MB transfer.
            nc.sync.dma_start(A48[:, 0:128], a_dram[:, 0:128])
            nc.sync.dma_start(B48[:, 0:512], b_dram[:, 0:512])
            nc.sync.dma_start(B48[:, 512:J], b_dram[:, 512:J])
            nc.sync.dma_start(A48[:, 128:1024], a_dram[:, 128:1024])
            nc.sync.dma_start(
                QA5[:].rearrange("p (t c) -> p t c", c=5),
                qa_dram[:].rearrange("(t p) c -> p t c", p=128),
            )
            nc.sync.dma_start(A48[:, 1024:3072], a_dram[:, 1024:3072])
            nc.sync.dma_start(A48[:, 3072:N], a_dram[:, 3072:N])

            acc = apool.tile([5, J], mybir.dt.float32)
            for t in range(NT):
                ts = slice(t * 128, (t + 1) * 128)
                g = gpool.tile([128, J], mybir.dt.float32)
                for j0, j1 in J_CHUNKS:
                    nc.tensor.matmul(
                        g[:, j0:j1], A48[:, ts], B48[:, j0:j1], start=True, stop=True
                    )
                k = kpool.tile([128, J], f16)
                nc.scalar.activation(k[:], g[:], mybir.ActivationFunctionType.Exp)
                cs = slice(t * 5, t * 5 + 5)
                for j0, j1 in J_CHUNKS:
                    nc.tensor.matmul(
                        acc[:, j0:j1], QA5[:, cs], k[:, j0:j1],
                        start=(t == 0), stop=(t == NT - 1),
                    )

            out_sb = opool.tile([5, J], mybir.dt.float32)
            nc.vector.tensor_copy(out_sb[:], acc[:])
            nc.sync.dma_start(out_dram[:], out_sb[:])

    nc.compile()
    return nc


_BUILDERS = {
    "fp32": _build_nc_fp32,
    "fp32r": _build_nc_fp32r,
    "bf16": _build_nc_bf16,
}


def _get_nc(scheme: str):
    nc = _BUILD_CACHE.get(scheme)
    if nc is None:
        nc = _BUILDERS[scheme]()
        _BUILD_CACHE[scheme] = nc
    return nc


def _round10(a):
    """Round fp32 array to 10-bit mantissa (round-to-nearest-even) = float32r grid."""
    u = np.asarray(a, dtype=np.float32).view(np.uint32)
    drop = 13  # keep 10 of 23 mantissa bits
    half = np.uint32(1 << (drop - 1))
    even = ((u >> drop) & np.uint32(1)).astype(np.uint32)
    u = u + half - np.uint32(1) + even
    u &= np.uint32(~((1 << drop) - 1) & 0xFFFFFFFF)
    return u.view(np.float32)


def _split10(a):
    hi = _round10(a)
    lo = _round10((a - hi).astype(np.float32))
    return hi, lo


def _split_bf16_3(a):
    import ml_dtypes

    bf = ml_dtypes.bfloat16
    a = np.asarray(a, dtype=np.float32)
    h = a.astype(bf)
    m = (a - h.astype(np.float32)).astype(bf)
    l = (a - h.astype(np.float32) - m.astype(np.float32)).astype(bf)
    return h, m, l


def _split_f16_2(a):
    a = np.asarray(a, dtype=np.float32)
    h = a.astype(np.float16)
    m = (a - h.astype(np.float32)).astype(np.float16)
    return h, m


def _host_prep(q_in, image, v_alpha, v_beta):
    """Augmented feature matrices (fp32, O(N) work only)."""
    q_in = np.asarray(q_in, dtype=np.float32)
    image = np.asarray(image, dtype=np.float32)
    v_alpha = np.asarray(v_alpha, dtype=np.float32)
    v_beta = np.asarray(v_beta, dtype=np.float32)

    z = np.arange(D, dtype=np.float32)[:, None, None]
    y = np.arange(H, dtype=np.float32)[None, :, None]
    x = np.arange(W, dtype=np.float32)[None, None, :]
    shp = (D, H, W)
    zz = np.broadcast_to(v_alpha[0] * z / np.float32(SIGMA_ALPHA[0]), shp)
    xx = np.broadcast_to(v_alpha[1] * x / np.float32(SIGMA_ALPHA[1]), shp)
    yy = np.broadcast_to(v_alpha[2] * y / np.float32(SIGMA_ALPHA[2]), shp)
    xyz = np.stack([zz, yy, xx], axis=3)
    rgb = v_beta * np.transpose(image, (1, 2, 3, 0)) / np.float32(SIGMA_BETA)
    feats = np.concatenate([xyz, rgb], axis=3).reshape(-1, 6).astype(np.float32)

    # Center each feature dim: |f_i - f_j| is translation invariant, smaller
    # magnitudes mean less cancellation in the PE accumulation.
    feats = feats - (feats.min(axis=0) + feats.max(axis=0)) * np.float32(0.5)

    s = np.einsum("nf,nf->n", feats, feats).astype(np.float32)

    a_all = np.empty((F, N), dtype=np.float32)
    a_all[0:6] = feats.T
    a_all[6] = -0.5 * s
    a_all[7] = 1.0

    b_full = np.empty((F, N), dtype=np.float32)
    b_full[0:6] = feats.T
    b_full[6] = 1.0
    b_full[7] = -0.5 * s

    qa = np.empty((N, 3), dtype=np.float32)
    qa[:, 0] = q_in[0].reshape(-1)
    qa[:, 1] = q_in[1].reshape(-1)
    qa[:, 2] = 1.0
    return a_all, b_full, qa


def _in_maps(scheme, a_all, b_full, qa):
    if scheme == "bf16":
        # K=48 stack [Ah;Ah;Ah;Am;Am;Al] . [Bh;Bm;Bl;Bh;Bm;Bh], zero-padded to
        # K=128: exact zeros in the accumulation, and NumWeights==128 keeps the
        # PE's fast-weight-load path enabled (K<128 forces slow LDWEIGHTS).
        import ml_dtypes

        ah, am, al = _split_bf16_3(a_all)
        bh, bm, bl = _split_bf16_3(b_full)
        zpad_a = np.zeros((128 - 6 * F, N), dtype=ml_dtypes.bfloat16)
        zpad_b = np.zeros((128 - 6 * F, b_full.shape[1]), dtype=ml_dtypes.bfloat16)
        a48 = np.concatenate([ah, ah, ah, am, am, al, zpad_a], axis=0)
        b48 = np.concatenate([bh, bm, bl, bh, bm, bh, zpad_b], axis=0)
        qh, qm = _split_f16_2(qa[:, 0:2])
        qa5 = np.empty((N, 5), dtype=np.float16)
        qa5[:, 0:2] = qh
        qa5[:, 2:4] = qm
        qa5[:, 4] = np.float16(1.0)
        return [
            {
                "a48": a48,
                "b48": np.ascontiguousarray(b48[:, c * J : (c + 1) * J]),
                "qa5": qa5,
            }
            for c in range(M_CORES)
        ]
    if scheme == "fp32r":
        a_hi, a_lo = _split10(a_all)
        b_hi, b_lo = _split10(b_full)
        qa_hi, qa_lo = _split10(qa)
        qa_lo[:, 2] = 0.0  # ones column lives entirely in the hi pass
        return [
            {
                "a_hi": a_hi,
                "a_lo": a_lo,
                "b_hi": np.ascontiguousarray(b_hi[:, c * J : (c + 1) * J]),
                "b_lo": np.ascontiguousarray(b_lo[:, c * J : (c + 1) * J]),
                "qa_hi": qa_hi,
                "qa_lo": qa_lo,
            }
            for c in range(M_CORES)
        ]
    return [
        {
            "a_all": a_all,
            "b_slab": np.ascontiguousarray(b_full[:, c * J : (c + 1) * J]),
            "qa": qa,
        }
        for c in range(M_CORES)
    ]


def kernel(q_in, image, v_alpha, v_beta):
    a_all, b_full, qa = _host_prep(q_in, image, v_alpha, v_beta)

    nc = _get_nc(_MM_SCHEME)
    in_maps = _in_maps(_MM_SCHEME, a_all, b_full, qa)
    res = run_bass_kernel_spmd(nc, in_maps, core_ids=list(range(M_CORES)))

    acc = np.concatenate([res.results[c]["acc_out"] for c in range(M_CORES)], axis=1)
    if _MM_SCHEME == "bf16":
        filtered = acc[0:2] + acc[2:4]
        norm = acc[4]
    else:
        filtered = acc[0:2]
        norm = acc[2]
    out = filtered / (norm[None, :] + EPS)
    return out.reshape(2, D, H, W).astype(np.float32)


# revision 20
# speedup vs baseline: 1.0329x; 1.0329x over previous
"""Trainium2 Bass kernel for nn_BilateralFilter (exact Gaussian bilateral filter).

Math (per reference):
  feats f_i in R^6 (scaled spatial zyx + scaled rgb), N = 12*24*24 = 6912
  sq[i,j] = |f_i - f_j|^2 ;  K = exp(-0.5*sq)
  out[c,j] = (sum_i q[c,i] K[i,j]) / (sum_i K[i,j] + eps)

Device strategy (8 cores, row-sharded over the N x N kernel):
  Each core owns J = N/8 = 864 output columns j.  One PE matmul per 128-i
  tile computes arg = f_i.f_j - 0.5|f_i|^2 - 0.5|f_j|^2 = -0.5*sq directly
  in PSUM via an augmented-feature contraction, ScalarE exponentiates
  PSUM->SBUF, and a second PE matmul contracts K against stacked
  [q_hi, q_lo, ones] columns accumulating (filtered_hi, filtered_lo, norm)
  in PSUM.  The N x N kernel matrix never touches HBM.

  Default scheme "bf16" keeps fp32-class accuracy on the fast 1-cycle/row
  PE path by stacking hi/lo splits into the contraction dimension:
  matmul1 is bf16 with 3-level operand splits (6 block-pairs, exact
  cross-products, dropped terms ~2^-27), zero-padded to K=128 so the PE's
  fast-weight-load path stays enabled; matmul2 is fp16 with 2-level q
  splits, and the fp16 rounding of K largely cancels between the filtered
  numerator and the norm.  Max rel err vs the fp32 reference: ~2.4e-5
  (BILATERAL_MM_SCHEME=fp32 selects the exact-fp32 path, ~1e-6, ~3x slower;
  fp32r selects a float32r 5-pass split variant, ~1e-5).

Host does only O(N) layout/prep: the augmented/split feature matrices and
the final (2 x N) hi+lo add + normalization divide (reference eps semantics).
"""

import os
import numpy as np

try:
    import concourse.bass as bass
except ImportError:  # fresh grading dir: repo not on sys.path
    import sys

    sys.path.insert(0, "/opt/trn_rl_repo")
    import concourse.bass as bass

import concourse.mybir as mybir
import concourse.tile as tile
from concourse import bacc
from concourse.bass_utils import run_bass_kernel_spmd

SIGMA_ALPHA = (5.0, 5.0, 5.0)
SIGMA_BETA = 0.3
EPS = float(np.finfo("float").eps)

D, H, W = 12, 24, 24
N = D * H * W  # 6912
M_CORES = 8
J = N // M_CORES  # 864 output columns per core
NT = N // 128  # 54 i-tiles
F = 8  # augmented feature dim
J_CHUNKS = [(0, 512), (512, 864)]  # matmul free-dim chunks, PSUM-bank aligned

# Schemes:
#   fp32  — exact fp32 matmuls (4 cycles/row on PE, 2 HW passes each)
#   fp32r — TF32-style float32r matmuls (1 cycle/row) with hi/lo 10-bit
#           operand splits so the math stays fp32-accurate:
#             matmul1: Ah.Bh + Ah.Bl + Al.Bh   (drops Al.Bl ~ 5e-6 in arg)
#             matmul2: [q_hi,1] pass + [q_lo,0] pass; the HW's 10-bit
#             truncation of K cancels between numerator and norm.
_MM_SCHEME = os.environ.get("BILATERAL_MM_SCHEME", "bf16")

_BUILD_CACHE: dict[str, object] = {}


def _build_nc_fp32():
    nc = bacc.Bacc(None, target_bir_lowering=False)

    a_dram = nc.dram_tensor("a_all", [F, N], mybir.dt.float32, kind="ExternalInput")
    b_dram = nc.dram_tensor("b_slab", [F, J], mybir.dt.float32, kind="ExternalInput")
    qa_dram = nc.dram_tensor("qa", [N, 3], mybir.dt.float32, kind="ExternalInput")
    out_dram = nc.dram_tensor("acc_out", [3, J], mybir.dt.float32, kind="ExternalOutput")

    with tile.TileContext(nc) as tc:
        with (
            tc.tile_pool(name="const", bufs=1) as const_pool,
            tc.tile_pool(name="kpool", bufs=4) as kpool,
            tc.tile_pool(name="gpsum", bufs=2, space="PSUM") as gpool,
            tc.tile_pool(name="apsum", bufs=1, space="PSUM") as apool,
            tc.tile_pool(name="opool", bufs=1) as opool,
        ):
            A = const_pool.tile([F, N], mybir.dt.float32)
            B = const_pool.tile([F, J], mybir.dt.float32)
            QA = const_pool.tile([128, NT * 3], mybir.dt.float32)
            nc.sync.dma_start(A[:], a_dram[:])
            nc.sync.dma_start(B[:], b_dram[:])
            nc.sync.dma_start(
                QA[:].rearrange("p (t c) -> p t c", c=3),
                qa_dram[:].rearrange("(t p) c -> p t c", p=128),
            )

            acc = apool.tile([3, J], mybir.dt.float32)
            for t in range(NT):
                g = gpool.tile([128, J], mybir.dt.float32)
                for j0, j1 in J_CHUNKS:
                    nc.tensor.matmul(
                        g[:, j0:j1],
                        A[:, t * 128 : (t + 1) * 128],
                        B[:, j0:j1],
                        start=True,
                        stop=True,
                    )
                k = kpool.tile([128, J], mybir.dt.float32)
                nc.scalar.activation(k[:], g[:], mybir.ActivationFunctionType.Exp)
                for j0, j1 in J_CHUNKS:
                    nc.tensor.matmul(
                        acc[:, j0:j1],
                        QA[:, t * 3 : t * 3 + 3],
                        k[:, j0:j1],
                        start=(t == 0),
                        stop=(t == NT - 1),
                    )

            out_sb = opool.tile([3, J], mybir.dt.float32)
            nc.vector.tensor_copy(out_sb[:], acc[:])
            nc.sync.dma_start(out_dram[:], out_sb[:])

    nc.compile()
    return nc


def _build_nc_fp32r():
    f32r = mybir.dt.float32r
    nc = bacc.Bacc(None, target_bir_lowering=False)

    ins = {}
    for name, shape in [
        ("a_hi", [F, N]),
        ("a_lo", [F, N]),
        ("b_hi", [F, J]),
        ("b_lo", [F, J]),
        ("qa_hi", [N, 3]),
        ("qa_lo", [N, 3]),
    ]:
        ins[name] = nc.dram_tensor(name, shape, f32r, kind="ExternalInput")
    out_dram = nc.dram_tensor("acc_out", [3, J], mybir.dt.float32, kind="ExternalOutput")

    with tile.TileContext(nc) as tc:
        with (
            tc.tile_pool(name="const", bufs=1) as const_pool,
            tc.tile_pool(name="kpool", bufs=4) as kpool,
            tc.tile_pool(name="gpsum", bufs=2, space="PSUM") as gpool,
            tc.tile_pool(name="apsum", bufs=1, space="PSUM") as apool,
            tc.tile_pool(name="opool", bufs=1) as opool,
        ):
            Ah = const_pool.tile([F, N], f32r)
            Al = const_pool.tile([F, N], f32r)
            Bh = const_pool.tile([F, J], f32r)
            Bl = const_pool.tile([F, J], f32r)
            QAh = const_pool.tile([128, NT * 3], f32r)
            QAl = const_pool.tile([128, NT * 3], f32r)
            nc.sync.dma_start(Ah[:], ins["a_hi"][:])
            nc.sync.dma_start(Al[:], ins["a_lo"][:])
            nc.sync.dma_start(Bh[:], ins["b_hi"][:])
            nc.sync.dma_start(Bl[:], ins["b_lo"][:])
            for t_sb, t_dr in [(QAh, "qa_hi"), (QAl, "qa_lo")]:
                nc.sync.dma_start(
                    t_sb[:].rearrange("p (t c) -> p t c", c=3),
                    ins[t_dr][:].rearrange("(t p) c -> p t c", p=128),
                )

            acc = apool.tile([3, J], mybir.dt.float32)
            for t in range(NT):
                ts = slice(t * 128, (t + 1) * 128)
                g = gpool.tile([128, J], mybir.dt.float32)
                # Ah.Bh, Ah.Bl with the same stationary operand, then Al.Bh.
                for j0, j1 in J_CHUNKS:
                    nc.tensor.matmul(g[:, j0:j1], Ah[:, ts], Bh[:, j0:j1], start=True, stop=False)
                for j0, j1 in J_CHUNKS:
                    nc.tensor.matmul(g[:, j0:j1], Ah[:, ts], Bl[:, j0:j1], start=False, stop=False)
                for j0, j1 in J_CHUNKS:
                    nc.tensor.matmul(g[:, j0:j1], Al[:, ts], Bh[:, j0:j1], start=False, stop=True)
                k = kpool.tile([128, J], f32r)
                nc.scalar.activation(k[:], g[:], mybir.ActivationFunctionType.Exp)
                cs = slice(t * 3, t * 3 + 3)
                for j0, j1 in J_CHUNKS:
                    nc.tensor.matmul(
                        acc[:, j0:j1], QAh[:, cs], k[:, j0:j1],
                        start=(t == 0), stop=False,
                    )
                for j0, j1 in J_CHUNKS:
                    nc.tensor.matmul(
                        acc[:, j0:j1], QAl[:, cs], k[:, j0:j1],
                        start=False, stop=(t == NT - 1),
                    )

            out_sb = opool.tile([3, J], mybir.dt.float32)
            nc.vector.tensor_copy(out_sb[:], acc[:])
            nc.sync.dma_start(out_dram[:], out_sb[:])

    nc.compile()
    return nc


def _build_nc_bf16():
    """Stacked-split scheme: matmul1 bf16 K=48, matmul2 fp16 M=5 — one MM per
    PSUM chunk, standalone LDWEIGHTS + FWL + back-to-back PE streaming."""
    bf16 = mybir.dt.bfloat16
    f16 = mybir.dt.float16
    nc = bacc.Bacc(None, target_bir_lowering=False)

    a_dram = nc.dram_tensor("a48", [128, N], bf16, kind="ExternalInput")
    b_dram = nc.dram_tensor("b48", [128, J], bf16, kind="ExternalInput")
    qa_dram = nc.dram_tensor("qa5", [N, 5], f16, kind="ExternalInput")
    out_dram = nc.dram_tensor("acc_out", [5, J], mybir.dt.float32, kind="ExternalOutput")

    with tile.TileContext(nc) as tc:
        with (
            tc.tile_pool(name="const", bufs=1) as const_pool,
            tc.tile_pool(name="kpool", bufs=4) as kpool,
            tc.tile_pool(name="gpsum", bufs=3, space="PSUM") as gpool,
            tc.tile_pool(name="apsum", bufs=1, space="PSUM") as apool,
            tc.tile_pool(name="opool", bufs=1) as opool,
        ):
            A48 = const_pool.tile([128, N], bf16)
            B48 = const_pool.tile([128, J], bf16)
            QA5 = const_pool.tile([128, NT * 5], f16)
            # Column-split the big A load so the first matmuls only wait for
            # the slice they need, not the full 1.8 MB transfer.  (Five
            # launches on one engine is the sweet spot: finer splits pay more
            # in serialized ~0.7us launch costs than they save in gating.)
            nc.sync.dma_start(B48[:], b_dram[:])
            nc.sync.dma_start(A48[:, 0:512], a_dram[:, 0:512])
            nc.sync.dma_start(
                QA5[:].rearrange("p (t c) -> p t c", c=5),
                qa_dram[:].rearrange("(t p) c -> p t c", p=128),
            )
            nc.sync.dma_start(A48[:, 512:2048], a_dram[:, 512:2048])
            nc.sync.dma_start(A48[:, 2048:N], a_dram[:, 2048:N])

            acc = apool.tile([5, J], mybir.dt.float32)
            for t in range(NT):
                ts = slice(t * 128, (t + 1) * 128)
                g = gpool.tile([128, J], mybir.dt.float32)
                for j0, j1 in J_CHUNKS:
                    nc.tensor.matmul(
                        g[:, j0:j1], A48[:, ts], B48[:, j0:j1], start=True, stop=True
                    )
                k = kpool.tile([128, J], f16)
                nc.scalar.activation(k[:], g[:], mybir.ActivationFunctionType.Exp)
                cs = slice(t * 5, t * 5 + 5)
                for j0, j1 in J_CHUNKS:
                    nc.tensor.matmul(
                        acc[:, j0:j1], QA5[:, cs], k[:, j0:j1],
                        start=(t == 0), stop=(t == NT - 1),
                    )

            out_sb = opool.tile([5, J], mybir.dt.float32)
            nc.vector.tensor_copy(out_sb[:], acc[:])
            nc.sync.dma_start(out_dram[:], out_sb[:])

    nc.compile()
    return nc


_BUILDERS = {
    "fp32": _build_nc_fp32,
    "fp32r": _build_nc_fp32r,
    "bf16": _build_nc_bf16,
}


def _get_nc(scheme: str):
    nc = _BUILD_CACHE.get(scheme)
    if nc is None:
        nc = _BUILDERS[scheme]()
        _BUILD_CACHE[scheme] = nc
    return nc


def _round10(a):
    """Round fp32 array to 10-bit mantissa (round-to-nearest-even) = float32r grid."""
    u = np.asarray(a, dtype=np.float32).view(np.uint32)
    drop = 13  # keep 10 of 23 mantissa bits
    half = np.uint32(1 << (drop - 1))
    even = ((u >> drop) & np.uint32(1)).astype(np.uint32)
    u = u + half - np.uint32(1) + even
    u &= np.uint32(~((1 << drop) - 1) & 0xFFFFFFFF)
    return u.view(np.float32)


def _split10(a):
    hi = _round10(a)
    lo = _round10((a - hi).astype(np.float32))
    return hi, lo


def _split_bf16_3(a):
    import ml_dtypes

    bf = ml_dtypes.bfloat16
    a = np.asarray(a, dtype=np.float32)
    h = a.astype(bf)
    m = (a - h.astype(np.float32)).astype(bf)
    l = (a - h.astype(np.float32) - m.astype(np.float32)).astype(bf)
    return h, m, l


def _split_f16_2(a):
    a = np.asarray(a, dtype=np.float32)
    h = a.astype(np.float16)
    m = (a - h.astype(np.float32)).astype(np.float16)
    return h, m


def _host_prep(q_in, image, v_alpha, v_beta):
    """Augmented feature matrices (fp32, O(N) work only)."""
    q_in = np.asarray(q_in, dtype=np.float32)
    image = np.asarray(image, dtype=np.float32)
    v_alpha = np.asarray(v_alpha, dtype=np.float32)
    v_beta = np.asarray(v_beta, dtype=np.float32)

    z = np.arange(D, dtype=np.float32)[:, None, None]
    y = np.arange(H, dtype=np.float32)[None, :, None]
    x = np.arange(W, dtype=np.float32)[None, None, :]
    shp = (D, H, W)
    zz = np.broadcast_to(v_alpha[0] * z / np.float32(SIGMA_ALPHA[0]), shp)
    xx = np.broadcast_to(v_alpha[1] * x / np.float32(SIGMA_ALPHA[1]), shp)
    yy = np.broadcast_to(v_alpha[2] * y / np.float32(SIGMA_ALPHA[2]), shp)
    xyz = np.stack([zz, yy, xx], axis=3)
    rgb = v_beta * np.transpose(image, (1, 2, 3, 0)) / np.float32(SIGMA_BETA)
    feats = np.concatenate([xyz, rgb], axis=3).reshape(-1, 6).astype(np.float32)

    # Center each feature dim: |f_i - f_j| is translation invariant, smaller
    # magnitudes mean less cancellation in the PE accumulation.
    feats = feats - (feats.min(axis=0) + feats.max(axis=0)) * np.float32(0.5)

    s = np.einsum("nf,nf->n", feats, feats).astype(np.float32)

    a_all = np.empty((F, N), dtype=np.float32)
    a_all[0:6] = feats.T
    a_all[6] = -0.5 * s
    a_all[7] = 1.0

    b_full = np.empty((F, N), dtype=np.float32)
    b_full[0:6] = feats.T
    b_full[6] = 1.0
    b_full[7] = -0.5 * s

    qa = np.empty((N, 3), dtype=np.float32)
    qa[:, 0] = q_in[0].reshape(-1)
    qa[:, 1] = q_in[1].reshape(-1)
    qa[:, 2] = 1.0
    return a_all, b_full, qa


def _in_maps(scheme, a_all, b_full, qa):
    if scheme == "bf16":
        # K=48 stack [Ah;Ah;Ah;Am;Am;Al] . [Bh;Bm;Bl;Bh;Bm;Bh], zero-padded to
        # K=128: exact zeros in the accumulation, and NumWeights==128 keeps the
        # PE's fast-weight-load path enabled (K<128 forces slow LDWEIGHTS).
        import ml_dtypes

        ah, am, al = _split_bf16_3(a_all)
        bh, bm, bl = _split_bf16_3(b_full)
        zpad_a = np.zeros((128 - 6 * F, N), dtype=ml_dtypes.bfloat16)
        zpad_b = np.zeros((128 - 6 * F, b_full.shape[1]), dtype=ml_dtypes.bfloat16)
        a48 = np.concatenate([ah, ah, ah, am, am, al, zpad_a], axis=0)
        b48 = np.concatenate([bh, bm, bl, bh, bm, bh, zpad_b], axis=0)
        qh, qm = _split_f16_2(qa[:, 0:2])
        qa5 = np.empty((N, 5), dtype=np.float16)
        qa5[:, 0:2] = qh
        qa5[:, 2:4] = qm
        qa5[:, 4] = np.float16(1.0)
        return [
            {
                "a48": a48,
                "b48": np.ascontiguousarray(b48[:, c * J : (c + 1) * J]),
                "qa5": qa5,
            }
            for c in range(M_CORES)
        ]
    if scheme == "fp32r":
        a_hi, a_lo = _split10(a_all)
        b_hi, b_lo = _split10(b_full)
        qa_hi, qa_lo = _split10(qa)
        qa_lo[:, 2] = 0.0  # ones column lives entirely in the hi pass
        return [
            {
                "a_hi": a_hi,
                "a_lo": a_lo,
                "b_hi": np.ascontiguousarray(b_hi[:, c * J : (c + 1) * J]),
                "b_lo": np.ascontiguousarray(b_lo[:, c * J : (c + 1) * J]),
                "qa_hi": qa_hi,
                "qa_lo": qa_lo,
            }
            for c in range(M_CORES)
        ]
    return [
        {
            "a_all": a_all,
            "b_slab": np.ascontiguousarray(b_full[:, c * J : (c + 1) * J]),
            "qa": qa,
        }
        for c in range(M_CORES)
    ]


def kernel(q_in, image, v_alpha, v_beta):
    a_all, b_full, qa = _host_prep(q_in, image, v_alpha, v_beta)

    nc = _get_nc(_MM_SCHEME)
    in_maps = _in_maps(_MM_SCHEME, a_all, b_full, qa)
    res = run_bass_kernel_spmd(nc, in_maps, core_ids=list(range(M_CORES)))

    acc = np.concatenate([res.results[c]["acc_out"] for c in range(M_CORES)], axis=1)
    if _MM_SCHEME == "bf16":
        filtered = acc[0:2] + acc[2:4]
        norm = acc[4]
    else:
        filtered = acc[0:2]
        norm = acc[2]
    out = filtered / (norm[None, :] + EPS)
    return out.reshape(2, D, H, W).astype(np.float32)


# revision 21
# speedup vs baseline: 1.0449x; 1.0116x over previous
"""Trainium2 Bass kernel for nn_BilateralFilter (exact Gaussian bilateral filter).

Math (per reference):
  feats f_i in R^6 (scaled spatial zyx + scaled rgb), N = 12*24*24 = 6912
  sq[i,j] = |f_i - f_j|^2 ;  K = exp(-0.5*sq)
  out[c,j] = (sum_i q[c,i] K[i,j]) / (sum_i K[i,j] + eps)

Device strategy (8 cores, row-sharded over the N x N kernel):
  Each core owns J = N/8 = 864 output columns j.  One PE matmul per 128-i
  tile computes arg = f_i.f_j - 0.5|f_i|^2 - 0.5|f_j|^2 = -0.5*sq directly
  in PSUM via an augmented-feature contraction, ScalarE exponentiates
  PSUM->SBUF, and a second PE matmul contracts K against stacked
  [q_hi, q_lo, ones] columns accumulating (filtered_hi, filtered_lo, norm)
  in PSUM.  The N x N kernel matrix never touches HBM.

  Default scheme "bf16" keeps fp32-class accuracy on the fast 1-cycle/row
  PE path by stacking hi/lo splits into the contraction dimension:
  matmul1 is bf16 with 3-level operand splits (6 block-pairs, exact
  cross-products, dropped terms ~2^-27), zero-padded to K=128 so the PE's
  fast-weight-load path stays enabled; matmul2 is fp16 with 2-level q
  splits, and the fp16 rounding of K largely cancels between the filtered
  numerator and the norm.  Max rel err vs the fp32 reference: ~2.4e-5
  (BILATERAL_MM_SCHEME=fp32 selects the exact-fp32 path, ~1e-6, ~3x slower;
  fp32r selects a float32r 5-pass split variant, ~1e-5).

Host does only O(N) layout/prep: the augmented/split feature matrices and
the final (2 x N) hi+lo add + normalization divide (reference eps semantics).
"""

import os
import numpy as np

try:
    import concourse.bass as bass
except ImportError:  # fresh grading dir: repo not on sys.path
    import sys

    sys.path.insert(0, "/opt/trn_rl_repo")
    import concourse.bass as bass

import concourse.mybir as mybir
import concourse.tile as tile
from concourse import bacc
from concourse.bass_utils import run_bass_kernel_spmd

SIGMA_ALPHA = (5.0, 5.0, 5.0)
SIGMA_BETA = 0.3
EPS = float(np.finfo("float").eps)

D, H, W = 12, 24, 24
N = D * H * W  # 6912
M_CORES = 8
J = N // M_CORES  # 864 output columns per core
NT = N // 128  # 54 i-tiles
F = 8  # augmented feature dim
J_CHUNKS = [(0, 512), (512, 864)]  # matmul free-dim chunks, PSUM-bank aligned

# Schemes:
#   fp32  — exact fp32 matmuls (4 cycles/row on PE, 2 HW passes each)
#   fp32r — TF32-style float32r matmuls (1 cycle/row) with hi/lo 10-bit
#           operand splits so the math stays fp32-accurate:
#             matmul1: Ah.Bh + Ah.Bl + Al.Bh   (drops Al.Bl ~ 5e-6 in arg)
#             matmul2: [q_hi,1] pass + [q_lo,0] pass; the HW's 10-bit
#             truncation of K cancels between numerator and norm.
_MM_SCHEME = os.environ.get("BILATERAL_MM_SCHEME", "bf16")

_BUILD_CACHE: dict[str, object] = {}


def _build_nc_fp32():
    nc = bacc.Bacc(None, target_bir_lowering=False)

    a_dram = nc.dram_tensor("a_all", [F, N], mybir.dt.float32, kind="ExternalInput")
    b_dram = nc.dram_tensor("b_slab", [F, J], mybir.dt.float32, kind="ExternalInput")
    qa_dram = nc.dram_tensor("qa", [N, 3], mybir.dt.float32, kind="ExternalInput")
    out_dram = nc.dram_tensor("acc_out", [3, J], mybir.dt.float32, kind="ExternalOutput")

    with tile.TileContext(nc) as tc:
        with (
            tc.tile_pool(name="const", bufs=1) as const_pool,
            tc.tile_pool(name="kpool", bufs=4) as kpool,
            tc.tile_pool(name="gpsum", bufs=2, space="PSUM") as gpool,
            tc.tile_pool(name="apsum", bufs=1, space="PSUM") as apool,
            tc.tile_pool(name="opool", bufs=1) as opool,
        ):
            A = const_pool.tile([F, N], mybir.dt.float32)
            B = const_pool.tile([F, J], mybir.dt.float32)
            QA = const_pool.tile([128, NT * 3], mybir.dt.float32)
            nc.sync.dma_start(A[:], a_dram[:])
            nc.sync.dma_start(B[:], b_dram[:])
            nc.sync.dma_start(
                QA[:].rearrange("p (t c) -> p t c", c=3),
                qa_dram[:].rearrange("(t p) c -> p t c", p=128),
            )

            acc = apool.tile([3, J], mybir.dt.float32)
            for t in range(NT):
                g = gpool.tile([128, J], mybir.dt.float32)
                for j0, j1 in J_CHUNKS:
                    nc.tensor.matmul(
                        g[:, j0:j1],
                        A[:, t * 128 : (t + 1) * 128],
                        B[:, j0:j1],
                        start=True,
                        stop=True,
                    )
                k = kpool.tile([128, J], mybir.dt.float32)
                nc.scalar.activation(k[:], g[:], mybir.ActivationFunctionType.Exp)
                for j0, j1 in J_CHUNKS:
                    nc.tensor.matmul(
                        acc[:, j0:j1],
                        QA[:, t * 3 : t * 3 + 3],
                        k[:, j0:j1],
                        start=(t == 0),
                        stop=(t == NT - 1),
                    )

            out_sb = opool.tile([3, J], mybir.dt.float32)
            nc.vector.tensor_copy(out_sb[:], acc[:])
            nc.sync.dma_start(out_dram[:], out_sb[:])

    nc.compile()
    return nc


def _build_nc_fp32r():
    f32r = mybir.dt.float32r
    nc = bacc.Bacc(None, target_bir_lowering=False)

    ins = {}
    for name, shape in [
        ("a_hi", [F, N]),
        ("a_lo", [F, N]),
        ("b_hi", [F, J]),
        ("b_lo", [F, J]),
        ("qa_hi", [N, 3]),
        ("qa_lo", [N, 3]),
    ]:
        ins[name] = nc.dram_tensor(name, shape, f32r, kind="ExternalInput")
    out_dram = nc.dram_tensor("acc_out", [3, J], mybir.dt.float32, kind="ExternalOutput")

    with tile.TileContext(nc) as tc:
        with (
            tc.tile_pool(name="const", bufs=1) as const_pool,
            tc.tile_pool(name="kpool", bufs=4) as kpool,
            tc.tile_pool(name="gpsum", bufs=2, space="PSUM") as gpool,
            tc.tile_pool(name="apsum", bufs=1, space="PSUM") as apool,
            tc.tile_pool(name="opool", bufs=1) as opool,
        ):
            Ah = const_pool.tile([F, N], f32r)
            Al = const_pool.tile([F, N], f32r)
            Bh = const_pool.tile([F, J], f32r)
            Bl = const_pool.tile([F, J], f32r)
            QAh = const_pool.tile([128, NT * 3], f32r)
            QAl = const_pool.tile([128, NT * 3], f32r)
            nc.sync.dma_start(Ah[:], ins["a_hi"][:])
            nc.sync.dma_start(Al[:], ins["a_lo"][:])
            nc.sync.dma_start(Bh[:], ins["b_hi"][:])
            nc.sync.dma_start(Bl[:], ins["b_lo"][:])
            for t_sb, t_dr in [(QAh, "qa_hi"), (QAl, "qa_lo")]:
                nc.sync.dma_start(
                    t_sb[:].rearrange("p (t c) -> p t c", c=3),
                    ins[t_dr][:].rearrange("(t p) c -> p t c", p=128),
                )

            acc = apool.tile([3, J], mybir.dt.float32)
            for t in range(NT):
                ts = slice(t * 128, (t + 1) * 128)
                g = gpool.tile([128, J], mybir.dt.float32)
                # Ah.Bh, Ah.Bl with the same stationary operand, then Al.Bh.
                for j0, j1 in J_CHUNKS:
                    nc.tensor.matmul(g[:, j0:j1], Ah[:, ts], Bh[:, j0:j1], start=True, stop=False)
                for j0, j1 in J_CHUNKS:
                    nc.tensor.matmul(g[:, j0:j1], Ah[:, ts], Bl[:, j0:j1], start=False, stop=False)
                for j0, j1 in J_CHUNKS:
                    nc.tensor.matmul(g[:, j0:j1], Al[:, ts], Bh[:, j0:j1], start=False, stop=True)
                k = kpool.tile([128, J], f32r)
                nc.scalar.activation(k[:], g[:], mybir.ActivationFunctionType.Exp)
                cs = slice(t * 3, t * 3 + 3)
                for j0, j1 in J_CHUNKS:
                    nc.tensor.matmul(
                        acc[:, j0:j1], QAh[:, cs], k[:, j0:j1],
                        start=(t == 0), stop=False,
                    )
                for j0, j1 in J_CHUNKS:
                    nc.tensor.matmul(
                        acc[:, j0:j1], QAl[:, cs], k[:, j0:j1],
                        start=False, stop=(t == NT - 1),
                    )

            out_sb = opool.tile([3, J], mybir.dt.float32)
            nc.vector.tensor_copy(out_sb[:], acc[:])
            nc.sync.dma_start(out_dram[:], out_sb[:])

    nc.compile()
    return nc


def _build_nc_bf16():
    """Stacked-split scheme: matmul1 bf16 K=48, matmul2 fp16 M=5 — one MM per
    PSUM chunk, standalone LDWEIGHTS + FWL + back-to-back PE streaming."""
    bf16 = mybir.dt.bfloat16
    f16 = mybir.dt.float16
    nc = bacc.Bacc(None, target_bir_lowering=False)

    a_dram = nc.dram_tensor("a48", [128, N], bf16, kind="ExternalInput")
    b_dram = nc.dram_tensor("b48", [128, J], bf16, kind="ExternalInput")
    qa_dram = nc.dram_tensor("qa5", [N, 5], f16, kind="ExternalInput")
    out_dram = nc.dram_tensor("acc_out", [5, J], mybir.dt.float32, kind="ExternalOutput")

    with tile.TileContext(nc) as tc:
        with (
            tc.tile_pool(name="const", bufs=1) as const_pool,
            tc.tile_pool(name="kpool", bufs=6) as kpool,
            tc.tile_pool(name="gpsum", bufs=3, space="PSUM") as gpool,
            tc.tile_pool(name="apsum", bufs=1, space="PSUM") as apool,
            tc.tile_pool(name="opool", bufs=1) as opool,
        ):
            A48 = const_pool.tile([128, N], bf16)
            B48 = const_pool.tile([128, J], bf16)
            QA5 = const_pool.tile([128, NT * 5], f16)
            # Column-split the big A load so the first matmuls only wait for
            # the slice they need, not the full 1.8 MB transfer.  (Five
            # launches on one engine is the sweet spot: finer splits pay more
            # in serialized ~0.7us launch costs than they save in gating.)
            nc.sync.dma_start(B48[:], b_dram[:])
            nc.sync.dma_start(A48[:, 0:512], a_dram[:, 0:512])
            nc.sync.dma_start(
                QA5[:].rearrange("p (t c) -> p t c", c=5),
                qa_dram[:].rearrange("(t p) c -> p t c", p=128),
            )
            nc.sync.dma_start(A48[:, 512:2048], a_dram[:, 512:2048])
            nc.sync.dma_start(A48[:, 2048:N], a_dram[:, 2048:N])

            acc = apool.tile([5, J], mybir.dt.float32)
            for t in range(NT):
                ts = slice(t * 128, (t + 1) * 128)
                g = gpool.tile([128, J], mybir.dt.float32)
                for j0, j1 in J_CHUNKS:
                    nc.tensor.matmul(
                        g[:, j0:j1], A48[:, ts], B48[:, j0:j1], start=True, stop=True
                    )
                k = kpool.tile([128, J], f16)
                nc.scalar.activation(k[:], g[:], mybir.ActivationFunctionType.Exp)
                cs = slice(t * 5, t * 5 + 5)
                for j0, j1 in J_CHUNKS:
                    nc.tensor.matmul(
                        acc[:, j0:j1], QA5[:, cs], k[:, j0:j1],
                        start=(t == 0), stop=(t == NT - 1),
                    )

            out_sb = opool.tile([5, J], mybir.dt.float32)
            nc.vector.tensor_copy(out_sb[:], acc[:])
            nc.sync.dma_start(out_dram[:], out_sb[:])

    nc.compile()
    return nc


_BUILDERS = {
    "fp32": _build_nc_fp32,
    "fp32r": _build_nc_fp32r,
    "bf16": _build_nc_bf16,
}


def _get_nc(scheme: str):
    nc = _BUILD_CACHE.get(scheme)
    if nc is None:
        nc = _BUILDERS[scheme]()
        _BUILD_CACHE[scheme] = nc
    return nc


def _round10(a):
    """Round fp32 array to 10-bit mantissa (round-to-nearest-even) = float32r grid."""
    u = np.asarray(a, dtype=np.float32).view(np.uint32)
    drop = 13  # keep 10 of 23 mantissa bits
    half = np.uint32(1 << (drop - 1))
    even = ((u >> drop) & np.uint32(1)).astype(np.uint32)
    u = u + half - np.uint32(1) + even
    u &= np.uint32(~((1 << drop) - 1) & 0xFFFFFFFF)
    return u.view(np.float32)


def _split10(a):
    hi = _round10(a)
    lo = _round10((a - hi).astype(np.float32))
    return hi, lo


def _split_bf16_3(a):
    import ml_dtypes

    bf = ml_dtypes.bfloat16
    a = np.asarray(a, dtype=np.float32)
    h = a.astype(bf)
    m = (a - h.astype(np.float32)).astype(bf)
    l = (a - h.astype(np.float32) - m.astype(np.float32)).astype(bf)
    return h, m, l


def _split_f16_2(a):
    a = np.asarray(a, dtype=np.float32)
    h = a.astype(np.float16)
    m = (a - h.astype(np.float32)).astype(np.float16)
    return h, m


def _host_prep(q_in, image, v_alpha, v_beta):
    """Augmented feature matrices (fp32, O(N) work only)."""
    q_in = np.asarray(q_in, dtype=np.float32)
    image = np.asarray(image, dtype=np.float32)
    v_alpha = np.asarray(v_alpha, dtype=np.float32)
    v_beta = np.asarray(v_beta, dtype=np.float32)

    z = np.arange(D, dtype=np.float32)[:, None, None]
    y = np.arange(H, dtype=np.float32)[None, :, None]
    x = np.arange(W, dtype=np.float32)[None, None, :]
    shp = (D, H, W)
    zz = np.broadcast_to(v_alpha[0] * z / np.float32(SIGMA_ALPHA[0]), shp)
    xx = np.broadcast_to(v_alpha[1] * x / np.float32(SIGMA_ALPHA[1]), shp)
    yy = np.broadcast_to(v_alpha[2] * y / np.float32(SIGMA_ALPHA[2]), shp)
    xyz = np.stack([zz, yy, xx], axis=3)
    rgb = v_beta * np.transpose(image, (1, 2, 3, 0)) / np.float32(SIGMA_BETA)
    feats = np.concatenate([xyz, rgb], axis=3).reshape(-1, 6).astype(np.float32)

    # Center each feature dim: |f_i - f_j| is translation invariant, smaller
    # magnitudes mean less cancellation in the PE accumulation.
    feats = feats - (feats.min(axis=0) + feats.max(axis=0)) * np.float32(0.5)

    s = np.einsum("nf,nf->n", feats, feats).astype(np.float32)

    a_all = np.empty((F, N), dtype=np.float32)
    a_all[0:6] = feats.T
    a_all[6] = -0.5 * s
    a_all[7] = 1.0

    b_full = np.empty((F, N), dtype=np.float32)
    b_full[0:6] = feats.T
    b_full[6] = 1.0
    b_full[7] = -0.5 * s

    qa = np.empty((N, 3), dtype=np.float32)
    qa[:, 0] = q_in[0].reshape(-1)
    qa[:, 1] = q_in[1].reshape(-1)
    qa[:, 2] = 1.0
    return a_all, b_full, qa


def _in_maps(scheme, a_all, b_full, qa):
    if scheme == "bf16":
        # K=48 stack [Ah;Ah;Ah;Am;Am;Al] . [Bh;Bm;Bl;Bh;Bm;Bh], zero-padded to
        # K=128: exact zeros in the accumulation, and NumWeights==128 keeps the
        # PE's fast-weight-load path enabled (K<128 forces slow LDWEIGHTS).
        import ml_dtypes

        ah, am, al = _split_bf16_3(a_all)
        bh, bm, bl = _split_bf16_3(b_full)
        zpad_a = np.zeros((128 - 6 * F, N), dtype=ml_dtypes.bfloat16)
        zpad_b = np.zeros((128 - 6 * F, b_full.shape[1]), dtype=ml_dtypes.bfloat16)
        a48 = np.concatenate([ah, ah, ah, am, am, al, zpad_a], axis=0)
        b48 = np.concatenate([bh, bm, bl, bh, bm, bh, zpad_b], axis=0)
        qh, qm = _split_f16_2(qa[:, 0:2])
        qa5 = np.empty((N, 5), dtype=np.float16)
        qa5[:, 0:2] = qh
        qa5[:, 2:4] = qm
        qa5[:, 4] = np.float16(1.0)
        return [
            {
                "a48": a48,
                "b48": np.ascontiguousarray(b48[:, c * J : (c + 1) * J]),
                "qa5": qa5,
            }
            for c in range(M_CORES)
        ]
    if scheme == "fp32r":
        a_hi, a_lo = _split10(a_all)
        b_hi, b_lo = _split10(b_full)
        qa_hi, qa_lo = _split10(qa)
        qa_lo[:, 2] = 0.0  # ones column lives entirely in the hi pass
        return [
            {
                "a_hi": a_hi,
                "a_lo": a_lo,
                "b_hi": np.ascontiguousarray(b_hi[:, c * J : (c + 1) * J]),
                "b_lo": np.ascontiguousarray(b_lo[:, c * J : (c + 1) * J]),
                "qa_hi": qa_hi,
                "qa_lo": qa_lo,
            }
            for c in range(M_CORES)
        ]
    return [
        {
            "a_all": a_all,
            "b_slab": np.ascontiguousarray(b_full[:, c * J : (c + 1) * J]),
            "qa": qa,
        }
        for c in range(M_CORES)
    ]


def kernel(q_in, image, v_alpha, v_beta):
    a_all, b_full, qa = _host_prep(q_in, image, v_alpha, v_beta)

    nc = _get_nc(_MM_SCHEME)
    in_maps = _in_maps(_MM_SCHEME, a_all, b_full, qa)
    res = run_bass_kernel_spmd(nc, in_maps, core_ids=list(range(M_CORES)))

    acc = np.concatenate([res.results[c]["acc_out"] for c in range(M_CORES)], axis=1)
    if _MM_SCHEME == "bf16":
        filtered = acc[0:2] + acc[2:4]
        norm = acc[4]
    else:
        filtered = acc[0:2]
        norm = acc[2]
    out = filtered / (norm[None, :] + EPS)
    return out.reshape(2, D, H, W).astype(np.float32)
